# revision 18
# baseline (speedup 1.0000x reference)
"""Trainium2 Bass kernel for out = x * exclusive_cumsum(x, axis=time).

Input x: [B=8, T=4096, D=1024] f32. Pure data parallel: batch element b -> core b.

Per-core algorithm (x_c: [T, D], partition axis = time), group-pipelined:
  - T is split into 32 blocks of 128 rows, processed as 4 groups of 8 blocks.
  - Per block: one fp16 cast (ACT) feeds both passes below.
  - Totals: per block b = 8g+i, a colsum matmul with selector weights (ones in
    lhsT slice-column i) accumulates the block's column totals into row i of a
    group PSUM tile [8, 512] per 512-wide D chunk; one DVE copy per group drops
    them into rows [32g : 32g+8] of a shared fp16 totals tile [128, 512]
    (quadrant-aligned bases 0/32/64/96; gap rows stay zero via memset).
  - Per block: a strict-upper-triangular 128x128 matmul computes the
    within-block exclusive cumsum into PSUM (start=True); a second matmul with
    lhsT = wcar[0:32g+i, :] (wcar[k,m] = 1 iff k mod 32 < 8, so exactly the
    totals of blocks < b are summed; gap rows hit zero weights) adds the carry
    to every partition (start=False). DVE/ACT multiply f32 x by the f32 PSUM
    prefix; the result DMAs out.
  - Group g's compute starts as soon as its own totals copy lands, overlapping
    later groups' loads: the PE never waits on a global phase boundary.

Scheduling refinements over the plain version (each A/B-benched on HW):
  - The first two x block loads lead the sync HWDGE ring (the three tiny
    weight loads follow them), so bulk-load descriptors start generating
    ~2us earlier and data flows at the fixed-preamble floor (~8.4us).
  - Stores are merged per block-PAIR per 512-chunk: one SWDGE instruction
    covers two blocks' chunk-j (out tile [128, 2, 1024], HBM dst rearranged
    "(q p) c -> p q c"), halving store instructions/semaphores, which
    shortens the end-of-kernel store-completion drain by ~2us. Every HBM
    run stays 2KB: >=4KB store descriptors intermittently made DMA engine
    15 a measured ~10-20% straggler (+7-16us). Stores issue from the
    (otherwise idle) GpSimd sequencer so they never head-of-line-block
    later loads on sync; the final pair's stores ride the (by then idle)
    sync HWDGE ring so the SWDGE queue drains earlier.

All bulk DMA is linear 512KB blocks at ~400 GB/s/core sustained (the
HBM-side ceiling with all 8 cores active; descriptor-size changes moved
per-engine busy time not at all). PE matmuls run in fp16 (1 cycle/row);
all accumulation stays fp32 in PSUM.
"""

import sys

sys.path.insert(0, "/opt/trn_rl_repo")

import numpy as np

B, T, D = 8, 4096, 1024
BLK = 128
NBLK = T // BLK      # 32
GRP = 8              # blocks per group
NGRP = NBLK // GRP   # 4
NCH = 2
CH = D // NCH        # 512, exactly one PSUM bank in f32

_CACHE = {}


def _weights(np_dtype=np.float16):
    wtri = np.triu(np.ones((BLK, BLK), dtype=np_dtype), 1)  # [k,m]=1 iff k<m
    # Selector: ones in column 64 only; wsel[:, 64-i : 72-i] has ones exactly
    # in slice-column i.
    wsel = np.zeros((BLK, BLK), dtype=np_dtype)
    wsel[:, 64] = 1.0
    # Carry weights: row k is all-ones iff it is a real totals row (k mod 32
    # < GRP); sliced to [0:32g+i, :] it sums exactly the totals of blocks < b.
    k = np.arange(BLK)[:, None]
    wcar = ((k % 32) < GRP).astype(np_dtype) * np.ones((1, BLK), dtype=np_dtype)
    return wtri, wsel, wcar


def build_nc(t=T, d=D, nch=NCH, num_devices=B, early_copies=True,
             store_group=2, weights_on_sync=True, last_store_on_sync=True,
             serial_stores=False):
    # early_copies: group-0 totals-prefix copies read finalized PSUM rows
    # while the accumulation group is still open. Verified correct on HW
    # (Tile orders copy_i between matmul_i and matmul_{i+1}; later matmuls
    # add exact zeros to rows <= i), but CoreSim forbids mid-group PSUM
    # reads, so the sim harness builds with early_copies=False.
    """Build the Bass module for one core's [t, d] shard."""
    import concourse.bass as bass
    import concourse.mybir as mybir
    import concourse.tile as tile
    from concourse import bacc

    f32 = mybir.dt.float32
    f16 = mybir.dt.float16
    ch = d // nch
    nblk = t // BLK
    ngrp = (nblk + GRP - 1) // GRP
    assert t % BLK == 0 and d % nch == 0 and ch <= 512 and nblk <= 32

    nc = bacc.Bacc("TRN2", target_bir_lowering=False, debug=False,
                   num_devices=num_devices)
    x = nc.dram_tensor("x", [t, d], f32, kind="ExternalInput").ap()
    wtri = nc.dram_tensor("wtri", [BLK, BLK], f16, kind="ExternalInput").ap()
    wsel = nc.dram_tensor("wsel", [BLK, BLK], f16, kind="ExternalInput").ap()
    wcar = nc.dram_tensor("wcar", [BLK, BLK], f16, kind="ExternalInput").ap()
    out = nc.dram_tensor("out", [t, d], f32, kind="ExternalOutput").ap()

    with tile.TileContext(nc) as tc:
        with (
            tc.tile_pool(name="wpool", bufs=1) as wpool,
            tc.tile_pool(name="xpool",
                         bufs=(t // BLK) if serial_stores else 16) as xpool,
            tc.tile_pool(name="hpool", bufs=12) as hpool,
            tc.tile_pool(name="spool", bufs=1) as spool,
            tc.tile_pool(name="opool",
                         bufs=(8 if serial_stores else 16)
                         // store_group) as opool,
            tc.tile_pool(name="ptot", bufs=1,
                         space=bass.MemorySpace.PSUM) as ptot,
            tc.tile_pool(name="pblk", bufs=3,
                         space=bass.MemorySpace.PSUM) as pblk,
        ):
            # First x block load leads the sync ring; weight loads ride the
            # (otherwise idle) scalar HWDGE ring so bulk-load descriptor
            # generation starts immediately.
            weng = nc.sync if weights_on_sync else nc.scalar
            xts_all = [None] * nblk
            # serial_stores: whole shard is SBUF-resident; emit every load up
            # front so the load stream is one uninterrupted read-only phase.
            nup = nblk if serial_stores else 2
            for b0 in range(2):
                xts_all[b0] = xpool.tile([BLK, d], f32, tag="xt",
                                         name=f"xt{b0}")
                nc.sync.dma_start(xts_all[b0][:],
                                  x[b0 * BLK:(b0 + 1) * BLK, :])

            wt = wpool.tile([BLK, BLK], f16, tag="wt")
            weng.dma_start(wt[:], wtri[:])
            ws = wpool.tile([BLK, BLK], f16, tag="ws")
            weng.dma_start(ws[:], wsel[:])
            wc = wpool.tile([BLK, BLK], f16, tag="wc")
            weng.dma_start(wc[:], wcar[:])

            for b0 in range(2, nup):
                xts_all[b0] = xpool.tile([BLK, d], f32, tag="xt",
                                         name=f"xt{b0}")
                nc.sync.dma_start(xts_all[b0][:],
                                  x[b0 * BLK:(b0 + 1) * BLK, :])
            gate = None
            if serial_stores:
                gate = wpool.tile([1, 8], f32, tag="gate")

            totals = []
            for j in range(nch):
                tj = spool.tile([BLK, ch], f16, tag=f"tots{j}",
                                name=f"totals{j}")
                nc.vector.memset(tj[:], 0.0)
                totals.append(tj)

            for g in range(ngrp):
                blo = g * GRP
                bhi = min(blo + GRP, nblk)
                nb = bhi - blo

                xts, xas = [], []
                tot_psum = []
                for j in range(nch):
                    tp = ptot.tile([nb, ch], f32, tag=f"totg{j}",
                                   name=f"totg{g}_{j}")
                    tot_psum.append(tp)
                for i in range(nb):
                    b = blo + i
                    if xts_all[b] is None:
                        xts_all[b] = xpool.tile([BLK, d], f32, tag="xt",
                                                name=f"xt{b}")
                        nc.sync.dma_start(xts_all[b][:],
                                          x[b * BLK:(b + 1) * BLK, :])
                    xt = xts_all[b]
                    xts.append(xt)
                    xa = hpool.tile([BLK, d], f16, tag="xa", name=f"xa{b}")
                    nc.scalar.copy(xa[:], xt[:])
                    xas.append(xa)
                    for j in range(nch):
                        jc = slice(j * ch, (j + 1) * ch)
                        nc.tensor.matmul(
                            tot_psum[j][:],
                            ws[:, 64 - i:64 - i + nb],  # slice-col i only
                            xa[:, jc],
                            start=(i == 0), stop=(i == nb - 1),
                        )
                        if early_copies and g == 0 and i < nb - 1:
                            # Early prefix copy: rows 0..i are final (later
                            # selector matmuls add exact zeros there), so
                            # block i+1's carry unblocks without waiting for
                            # the whole group. Startup-critical group 0 only:
                            # extending this to all groups was measured SLOWER
                            # (DVE congestion + totals-tile WAR ping-pong).
                            nc.vector.tensor_copy(
                                totals[j][0:i + 1, :],
                                tot_psum[j][0:i + 1, :])
                for j in range(nch):
                    nc.vector.tensor_copy(
                        totals[j][32 * g:32 * g + nb, :], tot_psum[j][:])

                for i in range(nb):
                    b = blo + i
                    kb = 32 * g + i  # totals rows covering blocks < b
                    q = i % store_group
                    if q == 0:
                        # One [128, store_group, d] out tile per block-GROUP
                        # -- partition p holds row p of blocks b..b+sg-1, so
                        # a single store instruction can cover sg blocks
                        # while every HBM run stays 2KB.
                        ot = opool.tile([BLK, store_group, d], f32,
                                        tag="out", name=f"ot{b}")
                    for j in range(nch):
                        jc = slice(j * ch, (j + 1) * ch)
                        ps = pblk.tile([BLK, ch], f32, tag=f"pb{j}",
                                       name=f"ps{b}_{j}")
                        nc.tensor.matmul(
                            ps[:], wt[:], xas[i][:, jc],
                            start=True, stop=(kb == 0),
                        )
                        if kb > 0:
                            nc.tensor.matmul(
                                ps[:],
                                wc[0:kb, :],         # rows k%32<8 are ones
                                totals[j][0:kb, :],
                                start=False, stop=True,
                            )
                        nc.any.tensor_mul(ot[:, q, jc], xts[i][:, jc],
                                          ps[:])
                        # Stores issue from the (otherwise idle) GpSimd
                        # sequencer so they never head-of-line-block later
                        # loads on sync. store_group merges sg blocks'
                        # chunk-j into ONE SWDGE instruction (fewer store
                        # semaphores shortens the end-of-kernel completion
                        # drain; HBM runs stay 2KB).
                        if q == store_group - 1:
                            sg = store_group
                            b0 = b - (sg - 1)
                            if sg == 1:
                                dst = out[b * BLK:(b + 1) * BLK, jc]
                                src = ot[:, 0, jc]
                            else:
                                dst = out[b0 * BLK:(b0 + sg) * BLK,
                                          jc].rearrange(
                                    "(q p) c -> p q c", q=sg)
                                src = ot[:, :, jc]
                            # The final block-pair's stores can ride the (by
                            # then idle) sync HWDGE ring: the SWDGE queue
                            # drains earlier and the last completion takes
                            # the faster HWDGE ack path.
                            if gate is not None:
                                # Order the whole store stream after the last
                                # load: the Pool queue is in-order, so one
                                # tiny copy reading xt[31] holds every store
                                # back until the read-only phase finishes
                                # (single-direction HBM streams avoid
                                # read/write turnaround).
                                nc.gpsimd.tensor_copy(
                                    gate[0:1, 0:1],
                                    xts_all[nblk - 1][0:1, 0:1])
                                gate = None
                            seng = (nc.sync if last_store_on_sync
                                    and b == nblk - 1 else nc.gpsimd)
                            seng.dma_start(dst, src)

    nc.compile()
    return nc


def kernel(x: np.ndarray) -> np.ndarray:
    from concourse.bass_utils import run_bass_kernel_spmd

    x = np.asarray(x, dtype=np.float32)
    assert x.shape == (B, T, D)
    key = "full"
    if key not in _CACHE:
        _CACHE[key] = build_nc()
    nc = _CACHE[key]

    wtri, wsel, wcar = _weights()
    in_maps = [
        {"x": np.ascontiguousarray(x[c]), "wtri": wtri, "wsel": wsel,
         "wcar": wcar}
        for c in range(B)
    ]
    res = run_bass_kernel_spmd(nc, in_maps, core_ids=list(range(B)))
    return np.stack([res.results[c]["out"] for c in range(B)], axis=0)
